# revision 21
# baseline (speedup 1.0000x reference)
# Differential multi-head attention (dual softmax + GroupNorm + sigmoid gating)
# for Trainium2, batch-parallel across 8 NeuronCores (one batch row per core).
#
# Per-core math (batch b):
#   q = query @ Wq + bq -> per head: q1, q2, gate (each S x 64)
#   k = key   @ Wk + bk -> per head: k1, k2
#   v = values@ Wv + bv -> per head: v (S x 64)
#   attn = softmax(q1 k1^T / 8) - lam * softmax(q2 k2^T / 8)
#   out  = GroupNorm_{8 groups over d, reduced over (S, heads, d-in-group)}(attn @ v)
#   out  = out * (1 - lambda_init) * sigmoid(gate)
#
# Layout strategy: d-major ("transposed") attention: scores are computed as
# s^T (k on partitions, q free) so the attn@v contraction runs at K=128, and
# exp row-sums come free via a ones-column appended to v (M=65).  q1/q2 (and
# k1/k2) of each head live in complementary 64-partition halves of one tile,
# so the two K=64 score matmuls of a head occupy disjoint PE row-groups and
# can run concurrently.  Matmul inputs are bf16 (single-pass PE); accumulation,
# softmax normalization, GroupNorm and the final output stay fp32.
# sigmoid(x) = (tanh(x/2)+1)/2 keeps ACT in one table set (exp/tanh/square).

import numpy as np

B, S_FULL, H, D = 8, 1024, 8, 64
DM = H * D  # 512


def build_nc(S=1024):
    import concourse.bacc as bacc
    import concourse.bass as bass
    import concourse.tile as tile
    from concourse import mybir
    from concourse.masks import make_identity

    f32 = mybir.dt.float32
    bf16 = mybir.dt.bfloat16
    AF = mybir.ActivationFunctionType
    OP = mybir.AluOpType
    AX = mybir.AxisListType

    NJ = S // 128          # k/seq 128-tiles
    CH = min(512, S)       # fp32-out matmul chunk
    NN = max(1, S // CH)
    CNT = float(S * H * (D // H))  # groupnorm reduction count per group
    EPS = 1e-3
    INV = 0.125            # 1/sqrt(64)

    nc = bacc.Bacc(target_bir_lowering=False)
    q_d = nc.dram_tensor("query", [S, DM], f32, kind="ExternalInput")
    k_d = nc.dram_tensor("key", [S, DM], f32, kind="ExternalInput")
    v_d = nc.dram_tensor("values", [S, DM], f32, kind="ExternalInput")
    wq_d = nc.dram_tensor("Wq", [DM, 3 * H * D], f32, kind="ExternalInput")
    bq_d = nc.dram_tensor("bq", [3 * H * D], f32, kind="ExternalInput")
    wk_d = nc.dram_tensor("Wk", [DM, 2 * H * D], f32, kind="ExternalInput")
    bk_d = nc.dram_tensor("bk", [2 * H * D], f32, kind="ExternalInput")
    wv_d = nc.dram_tensor("Wv", [DM, H * D], f32, kind="ExternalInput")
    bv_d = nc.dram_tensor("bv", [H * D], f32, kind="ExternalInput")
    gamma_d = nc.dram_tensor("gamma", [D], f32, kind="ExternalInput")
    beta_d = nc.dram_tensor("beta", [D], f32, kind="ExternalInput")
    lam_d = nc.dram_tensor("lam", [1], f32, kind="ExternalInput")
    li_d = nc.dram_tensor("lambda_init", [1], f32, kind="ExternalInput")
    out_d = nc.dram_tensor("out", [S, DM], f32, kind="ExternalOutput")

    ts_ = nc.vector.tensor_scalar
    stt = nc.vector.scalar_tensor_tensor

    with tile.TileContext(nc) as tc:
        with tc.tile_pool(name="consts", bufs=1) as consts, \
             tc.tile_pool(name="persist", bufs=1) as persist:

            # ---------- constants ----------
            ident = consts.tile([128, 128], f32, tag="ident", name="ident")
            make_identity(nc, ident)
            ident_b = consts.tile([128, 128], bf16, tag="ident_b", name="ident_b")
            make_identity(nc, ident_b)

            # block-diagonal group matrix: IND2[d', d] = 1 iff d'//8 == d//8
            ind2 = consts.tile([64, 64], f32, tag="ind2", name="ind2")
            nc.gpsimd.memset(ind2, 1.0)
            nc.gpsimd.affine_select(
                out=ind2, in_=ind2, compare_op=OP.is_ge, fill=0.0,
                base=0, pattern=[[-8, 8], [0, 8]], channel_multiplier=1)
            nc.gpsimd.affine_select(
                out=ind2, in_=ind2, compare_op=OP.is_ge, fill=0.0,
                base=7, pattern=[[8, 8], [0, 8]], channel_multiplier=-1)

            # selectors for the r-row broadcast matmul (per pair-half)
            # SP rows: [0]=sum1(even half), [1]=sum2, [2]=sum1(odd), [3]=sum2
            # sel[half][p, x] = 1 iff (x - 64p + 128*half) in [0, 64)
            sel = []
            for half in range(2):
                s_t = consts.tile([4, 128], f32, tag=f"sel{half}", name=f"sel{half}")
                nc.gpsimd.memset(s_t, 1.0)
                nc.gpsimd.affine_select(
                    out=s_t, in_=s_t, compare_op=OP.is_ge, fill=0.0,
                    base=128 * half, pattern=[[1, 128]], channel_multiplier=-64)
                nc.gpsimd.affine_select(
                    out=s_t, in_=s_t, compare_op=OP.is_ge, fill=0.0,
                    base=63 - 128 * half, pattern=[[-1, 128]], channel_multiplier=64)
                sel.append(s_t)

            # scalar columns
            lam64 = consts.tile([64, 1], f32, tag="lam64", name="lam64")
            nc.gpsimd.dma_start(out=lam64, in_=lam_d[:].to_broadcast([64, 1]))
            li64 = consts.tile([64, 1], f32, tag="li64", name="li64")
            nc.gpsimd.dma_start(out=li64, in_=li_d[:].to_broadcast([64, 1]))
            neglam64 = consts.tile([64, 1], f32, tag="neglam64", name="neglam64")
            ts_(neglam64, lam64, -1.0, None, OP.mult)
            onelam64 = consts.tile([64, 1], f32, tag="onelam64", name="onelam64")
            ts_(onelam64, lam64, -1.0, 1.0, OP.mult, OP.add)   # 1 - lam
            halfli = consts.tile([64, 1], f32, tag="halfli", name="halfli")
            ts_(halfli, li64, -0.5, 0.5, OP.mult, OP.add)      # 0.5*(1-li)

            gamma_c = consts.tile([64, 1], f32, tag="gamma_c", name="gamma_c")
            nc.sync.dma_start(out=gamma_c, in_=gamma_d[:])
            beta_c = consts.tile([64, 1], f32, tag="beta_c", name="beta_c")
            nc.sync.dma_start(out=beta_c, in_=beta_d[:])
            bb64 = consts.tile([64, 1], f32, tag="bb64", name="bb64")
            ts_(bb64, beta_c, halfli, None, OP.mult)           # beta*0.5*(1-li)

            # v-bias columns per head and C = bv*(1-lam) fold
            bvc = consts.tile([64, 8], f32, tag="bvc", name="bvc")
            nc.sync.dma_start(
                out=bvc, in_=bv_d[:].rearrange("(h d) -> d h", d=64))
            cc = consts.tile([64, 8], f32, tag="cc", name="cc")
            ts_(cc, bvc, onelam64, None, OP.mult)

            # bias columns: per-head stacked [q1|q2] / [k1|k2] are contiguous
            # 128-element runs of bq/bk; gate needs a gathered layout.
            bqp = consts.tile([128, 8], f32, tag="bqp", name="bqp")
            nc.sync.dma_start(
                out=bqp,
                in_=bq_d[:].rearrange("(h blk) -> blk h", blk=192)[0:128, :])
            bkp = consts.tile([128, 8], f32, tag="bkp", name="bkp")
            nc.sync.dma_start(
                out=bkp,
                in_=bk_d[:].rearrange("(h blk) -> blk h", blk=128))
            bg = consts.tile([128, 4], f32, tag="bg", name="bg")
            bqv = bq_d[:].rearrange("(h blk) -> h blk", blk=192)
            for p in range(4):
                nc.sync.dma_start(out=bg[:, p:p + 1],
                                  in_=bqv[2 * p:2 * p + 2, 128:192])

            # persistent projection outputs (bf16, d-major)
            # qp/kp[h]: rows 0-63 = q1/k1 of head h, rows 64-127 = q2/k2
            qp = [persist.tile([128, S], bf16, tag=f"qp{h}", name=f"qp{h}") for h in range(8)]
            # zero-padded key tiles: kz1[h] rows 0-63 = k1 (rest 0),
            # kz2[h] rows 64-127 = k2 (rest 0) -> K=128 score matmuls
            kz1 = [persist.tile([128, S], bf16, tag=f"kz1{h}", name=f"kz1{h}") for h in range(8)]
            kz2 = [persist.tile([128, S], bf16, tag=f"kz2{h}", name=f"kz2{h}") for h in range(8)]
            for h in range(8):
                nc.vector.memset(kz1[h][64:128, :], 0.0)
                nc.vector.memset(kz2[h][0:64, :], 0.0)
            # gate stays head-pair packed: gt[p] rows 0-63 = head 2p, 64-127 = 2p+1
            gt = [persist.tile([128, S], bf16, tag=f"gt{p}", name=f"gt{p}") for p in range(4)]
            va = [persist.tile([128, 8, 65], bf16, tag=f"va{i}", name=f"va{i}") for i in range(NJ)]
            ypair = [persist.tile([128, S], f32, tag=f"yp{p}", name=f"yp{p}") for p in range(4)]
            sumcol = persist.tile([64, 16], f32, tag="sumcol", name="sumcol")

            # ---------- phase 1: load + transpose inputs (DMA only) ----------
            # fp32 DRAM -> (cast DMA) -> bf16 DRAM scratch -> (xbar transpose
            # DMA) -> x^T bf16 in SBUF, 4 tiles of (128, S) per tensor.
            GRP = min(4, NJ)
            with tc.tile_pool(name="xin", bufs=3) as xin_pool, \
                 tc.tile_pool(name="xtp", bufs=1) as xtp, \
                 tc.tile_pool(name="wload", bufs=1) as wpool, \
                 tc.tile_pool(name="ps_in", bufs=1, space="PSUM") as ps_in, \
                 tc.tile_pool(name="ps_proj", bufs=4, space="PSUM") as ps_proj:

                def transpose_input(x_dram, nm):
                    xt = [xtp.tile([128, S], bf16, tag=f"xt{nm}{c}", name=f"xt{nm}{c}")
                          for c in range(4)]
                    tp_cur = [None] * 4
                    for i in range(NJ):
                        xs = xin_pool.tile([128, DM], f32, tag="xs", name="xs")
                        nc.sync.dma_start(out=xs, in_=x_dram[128 * i:128 * (i + 1), :])
                        xq = xin_pool.tile([128, DM], bf16, tag="xin", name="xin")
                        nc.vector.tensor_copy(xq, xs)
                        if i % GRP == 0:
                            for c in range(4):
                                tp_cur[c] = ps_in.tile(
                                    [128, 128 * GRP], bf16, tag=f"tp{c}", name=f"tp{c}")
                        for c in range(4):
                            nc.tensor.transpose(
                                tp_cur[c][:, 128 * (i % GRP):128 * (i % GRP + 1)],
                                xq[:, 128 * c:128 * (c + 1)], ident_b)
                        if i % GRP == GRP - 1:
                            base = 128 * GRP * (i // GRP)
                            for c in range(4):
                                nc.vector.tensor_copy(
                                    xt[c][:, base:base + 128 * GRP], tp_cur[c])
                    return xt

                # --- query path: qp[h] then gate ---
                # (x casts issue first so the SWDGE queue isn't stuck behind
                # the 6MB of weight casts at kernel start)
                xtq = transpose_input(q_d, "q")
                wqf = [wpool.tile([128, 3 * H * D], bf16, tag=f"wqf{r}", name=f"wqf{r}") for r in range(4)]
                wkf = [wpool.tile([128, 2 * H * D], bf16, tag=f"wkf{r}", name=f"wkf{r}") for r in range(4)]
                wvf = [wpool.tile([128, H * D], bf16, tag=f"wvf{r}", name=f"wvf{r}") for r in range(4)]
                # stage fp32 weights via HWDGE (fast, parallel to the x casts
                # on the SWDGE queue), downcast on the otherwise-idle ACT
                for r in range(4):
                    wsq = wpool.tile([128, 3 * H * D], f32, tag=f"wsq{r}", name=f"wsq{r}")
                    nc.sync.dma_start(out=wsq, in_=wq_d[128 * r:128 * (r + 1), :])
                    nc.scalar.copy(wqf[r], wsq)
                for r in range(4):
                    wsk = wpool.tile([128, 2 * H * D], f32, tag=f"wsk{r}", name=f"wsk{r}")
                    nc.sync.dma_start(out=wsk, in_=wk_d[128 * r:128 * (r + 1), :])
                    nc.scalar.copy(wkf[r], wsk)
                    wsv = wpool.tile([128, H * D], f32, tag=f"wsv{r}", name=f"wsv{r}")
                    nc.sync.dma_start(out=wsv, in_=wv_d[128 * r:128 * (r + 1), :])
                    nc.scalar.copy(wvf[r], wsv)
                for h in range(8):
                    for n in range(NN):
                        ps = ps_proj.tile([128, CH], f32, tag="proj", name="proj")
                        for r in range(4):
                            nc.tensor.matmul(
                                ps, wqf[r][:, 192 * h:192 * h + 128],
                                xtq[r][:, CH * n:CH * (n + 1)],
                                start=(r == 0), stop=(r == 3))
                        nc.scalar.activation(
                            qp[h][:, CH * n:CH * (n + 1)], ps, AF.Identity,
                            bias=bqp[:, h:h + 1])
                # gate: pre-gathered pair-packed weight tiles (the 64-col
                # blocks of heads 2p/2p+1 collected by the load DMA)
                wgt = []
                for r in range(4):
                    w_t = wpool.tile([128, 512], bf16, tag=f"wg{r}", name=f"wg{r}")
                    nc.gpsimd.dma_start(
                        out=w_t,
                        in_=wq_d[128 * r:128 * (r + 1), :].rearrange(
                            "k (h blk) -> k h blk", blk=192)[:, :, 128:192])
                    wgt.append(w_t)
                for p in range(4):
                    for n in range(NN):
                        ps = ps_proj.tile([128, CH], f32, tag="proj", name="proj")
                        for r in range(4):
                            nc.tensor.matmul(
                                ps, wgt[r][:, 128 * p:128 * (p + 1)],
                                xtq[r][:, CH * n:CH * (n + 1)],
                                start=(r == 0), stop=(r == 3))
                        nc.scalar.activation(
                            gt[p][:, CH * n:CH * (n + 1)], ps, AF.Identity,
                            bias=bg[:, p:p + 1])

                # --- key path ---
                xtk = transpose_input(k_d, "k")
                for h in range(8):
                    for n in range(NN):
                        ps = ps_proj.tile([128, CH], f32, tag="proj", name="proj")
                        for r in range(4):
                            nc.tensor.matmul(
                                ps, wkf[r][:, 128 * h:128 * (h + 1)],
                                xtk[r][:, CH * n:CH * (n + 1)],
                                start=(r == 0), stop=(r == 3))
                        nc.scalar.activation(
                            kz1[h][0:64, CH * n:CH * (n + 1)], ps[0:64, :],
                            AF.Identity, bias=bkp[0:64, h:h + 1])
                        nc.scalar.activation(
                            kz2[h][64:128, CH * n:CH * (n + 1)], ps[64:128, :],
                            AF.Identity, bias=bkp[64:128, h:h + 1])

                # --- values path (q-major, interleaved into v_aug + ones) ---
                xtv = transpose_input(v_d, "v")
                for i in range(NJ):
                    ps = ps_proj.tile([128, 512], f32, tag="proj", name="proj")
                    for r in range(4):
                        nc.tensor.matmul(
                            ps, xtv[r][:, 128 * i:128 * (i + 1)], wvf[r],
                            start=(r == 0), stop=(r == 3))
                    nc.scalar.copy(
                        va[i][:, :, 0:64],
                        ps.rearrange("p (h d) -> p h d", d=64))
                    nc.gpsimd.memset(va[i][:, :, 64:65], 1.0)

                # gate tanh now (ACT is free here; result only needed at the
                # very end) -- th_t lives in the persist pool
                th_t = [persist.tile([128, S], f32, tag=f"th{p}", name=f"th{p}")
                        for p in range(4)]
                for p in range(4):
                    nc.scalar.activation(th_t[p], gt[p], AF.Tanh, scale=0.5)

            # ---------- phase 2: attention per head (pairs for epilogue) ----
            with tc.tile_pool(name="ps_att", bufs=1, space="PSUM") as ps_att, \
                 tc.tile_pool(name="ps_o", bufs=1, space="PSUM") as ps_o, \
                 tc.tile_pool(name="expp", bufs=3) as expp, \
                 tc.tile_pool(name="osp", bufs=2) as osp, \
                 tc.tile_pool(name="spp", bufs=2) as spp:

                def emit_combine(sp, os_t, p):
                    rp = spp.tile([4, S], f32, tag="rp", name="rp", bufs=1)
                    rscr = spp.tile([4, S], f32, tag="rscr", name="rscr", bufs=1)
                    nc.vector.reciprocal_approx_accurate(rp, sp, rscr)
                    rst = []
                    for i in range(4):
                        r_t = spp.tile([1, S], f32, tag=f"rst{i}", name=f"rst{i}", bufs=1)
                        nc.sync.dma_start(out=r_t, in_=rp[i:i + 1, :])
                        rst.append(r_t)
                    for half in range(2):
                        h = 2 * p + half
                        os1, os2 = os_t[half]
                        bcs1 = spp.tile([64, S], f32, tag="bcs1", name="bcs1", bufs=1)
                        bcs2 = spp.tile([64, S], f32, tag="bcs2", name="bcs2", bufs=1)
                        nc.gpsimd.partition_broadcast(
                            bcs1, rst[2 * half][0:1, :], channels=64)
                        nc.gpsimd.partition_broadcast(
                            bcs2, rst[2 * half + 1][0:1, :], channels=64)
                        nc.vector.tensor_mul(os1[0:64, :], os1[0:64, :], bcs1)
                        stt(os2[0:64, :], os2[0:64, :], neglam64, bcs2,
                            OP.mult, OP.mult)
                        ydst = ypair[p][64 * half:64 * half + 64, :]
                        stt(ydst, os1[0:64, :], 1.0, os2[0:64, :],
                            OP.bypass, OP.add, accum_out=sumcol[:, h:h + 1])
                        stt(os1[0:64, :], ydst, 1.0, ydst, OP.mult, OP.mult,
                            accum_out=sumcol[:, 8 + h:9 + h])

                for p in range(4):
                    os_t = {}
                    sp = spp.tile([4, S], f32, tag="sp", name="sp")
                    for half in range(2):
                        h = 2 * p + half
                        o_ps = {}
                        for t in (1, 2):
                            o_ps[t] = ps_o.tile([65, S], f32, tag=f"o{t}", name=f"o{t}")
                        for j in range(NJ):
                            for t, kz_ in ((1, kz1), (2, kz2)):
                                s_ps = ps_att.tile([128, S], f32, tag=f"s{t}", name=f"s{t}")
                                for n in range(NN):
                                    nc.tensor.matmul(
                                        s_ps[:, CH * n:CH * (n + 1)],
                                        kz_[h][:, 128 * j:128 * (j + 1)],
                                        qp[h][:, CH * n:CH * (n + 1)],
                                        start=True, stop=True)
                                ex = expp.tile([128, S], bf16, tag=f"exp{t}", name=f"exp{t}")
                                nc.scalar.activation(ex, s_ps, AF.Exp, scale=INV)
                                for n in range(NN):
                                    nc.tensor.matmul(
                                        o_ps[t][:, CH * n:CH * (n + 1)],
                                        va[j][:, h, :],
                                        ex[:, CH * n:CH * (n + 1)],
                                        start=(j == 0), stop=(j == NJ - 1))
                        os1 = osp.tile([65, S], f32, tag=f"os1_{half}", name=f"os1_{half}")
                        os2 = osp.tile([65, S], f32, tag=f"os2_{half}", name=f"os2_{half}")
                        nc.vector.tensor_copy(os1, o_ps[1])
                        nc.vector.tensor_copy(os2, o_ps[2])
                        os_t[half] = (os1, os2)
                        nc.sync.dma_start(out=sp[2 * half:2 * half + 1, :],
                                          in_=os1[64:65, :])
                        nc.sync.dma_start(out=sp[2 * half + 1:2 * half + 2, :],
                                          in_=os2[64:65, :])
                    emit_combine(sp, os_t, p)

            # ---------- phase 3: stats, groupnorm, gate, output ----------
            with tc.tile_pool(name="tailp", bufs=1) as tailp, \
                 tc.tile_pool(name="oq", bufs=3) as oqp, \
                 tc.tile_pool(name="ps_tail", bufs=2, space="PSUM") as ps_tail:

                tot = tailp.tile([64, 2], f32, tag="tot", name="tot")
                nc.vector.tensor_reduce(
                    tot, sumcol.rearrange("p (t h) -> p t h", h=8),
                    axis=AX.X, op=OP.add)
                # bias-C (bv) corrections to the raw-Y stats
                csc = tailp.tile([64, 8], f32, tag="csc", name="csc")
                nc.vector.tensor_mul(csc, cc, sumcol[:, 0:8])
                cy64 = tailp.tile([64, 1], f32, tag="cy64", name="cy64")
                nc.vector.tensor_reduce(cy64, csc, axis=AX.X, op=OP.add)
                nc.vector.tensor_mul(csc, cc, cc)
                csq64 = tailp.tile([64, 1], f32, tag="csq64", name="csq64")
                nc.vector.tensor_reduce(csq64, csc, axis=AX.X, op=OP.add)
                csum64 = tailp.tile([64, 1], f32, tag="csum64", name="csum64")
                nc.vector.tensor_reduce(csum64, cc, axis=AX.X, op=OP.add)
                tot2 = tailp.tile([64, 2], f32, tag="tot2", name="tot2")
                stt(tot2[:, 0:1], csum64, float(S), tot[:, 0:1], OP.mult, OP.add)
                stt(tot2[:, 1:2], cy64, 2.0, tot[:, 1:2], OP.mult, OP.add)
                stt(tot2[:, 1:2], csq64, float(S), tot2[:, 1:2], OP.mult, OP.add)

                ms_ps = ps_tail.tile([64, 2], f32, tag="ms", name="ms")
                nc.tensor.matmul(ms_ps, ind2, tot2, start=True, stop=True)
                mean64 = tailp.tile([64, 1], f32, tag="mean64", name="mean64")
                ts_(mean64, ms_ps[:, 0:1], 1.0 / CNT, None, OP.mult)
                e264 = tailp.tile([64, 1], f32, tag="e264", name="e264")
                ts_(e264, ms_ps[:, 1:2], 1.0 / CNT, None, OP.mult)
                nm2 = tailp.tile([64, 1], f32, tag="nm2", name="nm2")
                ts_(nm2, mean64, mean64, -1.0, OP.mult, OP.mult)
                veps = tailp.tile([64, 1], f32, tag="veps", name="veps")
                stt(veps, nm2, EPS, e264, OP.add, OP.add)
                sd = tailp.tile([64, 1], f32, tag="sd", name="sd")
                nc.scalar.activation(sd, veps, AF.Sqrt)
                rsd = tailp.tile([64, 1], f32, tag="rsd", name="rsd")
                nc.vector.reciprocal(rsd, sd)
                # one Newton step for rsqrt accuracy (ACT sqrt is loose)
                rr = tailp.tile([64, 1], f32, tag="rr", name="rr")
                nc.vector.tensor_mul(rr, rsd, rsd)
                nc.vector.tensor_mul(rr, rr, veps)
                ts_(rr, rr, -0.5, 1.5, OP.mult, OP.add)
                rstd = tailp.tile([64, 1], f32, tag="rstd", name="rstd")
                nc.vector.tensor_mul(rstd, rsd, rr)

                a64 = tailp.tile([64, 1], f32, tag="a64", name="a64")
                ts_(a64, rstd, gamma_c, halfli, OP.mult, OP.mult)
                cm = tailp.tile([64, 8], f32, tag="cm", name="cm")
                ts_(cm, cc, mean64, None, OP.subtract)
                ball = tailp.tile([64, 8], f32, tag="ball", name="ball")
                ts_(ball, cm, a64, bb64, OP.mult, OP.add)

                for p in range(4):
                    for half in range(2):
                        h = 2 * p + half
                        rows = ypair[p][64 * half:64 * half + 64, :]
                        nc.scalar.activation(rows, rows, AF.Identity,
                                             bias=ball[:, h:h + 1], scale=a64)
                    stt(ypair[p], th_t[p], 1.0, ypair[p], OP.add, OP.mult)

                for c in range(NJ):
                    tp_o = ps_tail.tile([128, 512], f32, tag="tp_out", name="tp_out")
                    for p in range(4):
                        nc.tensor.transpose(
                            tp_o[:, 128 * p:128 * (p + 1)],
                            ypair[p][:, 128 * c:128 * (c + 1)], ident)
                    oq = oqp.tile([128, 512], f32, tag="oq", name="oq")
                    nc.scalar.copy(oq, tp_o)
                    nc.sync.dma_start(out=out_d[128 * c:128 * (c + 1), :], in_=oq)

    nc.finalize()
    return nc


_CACHE = {}


def _get_nc():
    if "nc" not in _CACHE:
        _CACHE["nc"] = build_nc(S_FULL)
    return _CACHE["nc"]


def run(inputs, trace=False, tmpdir=None):
    from concourse.bass_utils import run_bass_kernel_spmd
    nc = _get_nc()
    arrs = {k: np.asarray(v, dtype=np.float32) for k, v in inputs.items()}
    shared = {k: np.ascontiguousarray(arrs[k]) for k in
              ("Wq", "bq", "Wk", "bk", "Wv", "bv", "gamma", "beta",
               "lam", "lambda_init")}
    in_maps = []
    for i in range(B):
        m = dict(shared)
        m["query"] = np.ascontiguousarray(arrs["query"][i])
        m["key"] = np.ascontiguousarray(arrs["key"][i])
        m["values"] = np.ascontiguousarray(arrs["values"][i])
        in_maps.append(m)
    res = run_bass_kernel_spmd(nc, in_maps, core_ids=list(range(B)),
                               trace=trace, tmpdir=tmpdir)
    out = np.stack([res.results[i]["out"] for i in range(B)], axis=0)
    return out.astype(np.float32), res


def kernel(**inputs):
    out, _ = run(inputs)
    return out


# revision 22
# speedup vs baseline: 1.1629x; 1.1629x over previous
# Differential multi-head attention (dual softmax + GroupNorm + sigmoid gating)
# for Trainium2, batch-parallel across 8 NeuronCores (one batch row per core).
#
# Per-core math (batch b):
#   q = query @ Wq + bq -> per head: q1, q2, gate (each S x 64)
#   k = key   @ Wk + bk -> per head: k1, k2
#   v = values@ Wv + bv -> per head: v (S x 64)
#   attn = softmax(q1 k1^T / 8) - lam * softmax(q2 k2^T / 8)
#   out  = GroupNorm_{8 groups over d, reduced over (S, heads, d-in-group)}(attn @ v)
#   out  = out * (1 - lambda_init) * sigmoid(gate)
#
# Layout strategy: d-major ("transposed") attention: scores are computed as
# s^T (k on partitions, q free) so the attn@v contraction runs at K=128, and
# exp row-sums come free via a ones-column appended to v (M=65).  q1/q2 (and
# k1/k2) of each head live in complementary 64-partition halves of one tile,
# so the two K=64 score matmuls of a head occupy disjoint PE row-groups and
# can run concurrently.  Matmul inputs are bf16 (single-pass PE); accumulation,
# softmax normalization, GroupNorm and the final output stay fp32.
# sigmoid(x) = (tanh(x/2)+1)/2 keeps ACT in one table set (exp/tanh/square).

import numpy as np

B, S_FULL, H, D = 8, 1024, 8, 64
DM = H * D  # 512


def build_nc(S=1024):
    import concourse.bacc as bacc
    import concourse.bass as bass
    import concourse.tile as tile
    from concourse import mybir
    from concourse.masks import make_identity

    f32 = mybir.dt.float32
    bf16 = mybir.dt.bfloat16
    AF = mybir.ActivationFunctionType
    OP = mybir.AluOpType
    AX = mybir.AxisListType

    NJ = S // 128          # k/seq 128-tiles
    CH = min(512, S)       # fp32-out matmul chunk
    NN = max(1, S // CH)
    CNT = float(S * H * (D // H))  # groupnorm reduction count per group
    EPS = 1e-3
    INV = 0.125            # 1/sqrt(64)

    nc = bacc.Bacc(target_bir_lowering=False)
    q_d = nc.dram_tensor("query", [S, DM], f32, kind="ExternalInput")
    k_d = nc.dram_tensor("key", [S, DM], f32, kind="ExternalInput")
    v_d = nc.dram_tensor("values", [S, DM], f32, kind="ExternalInput")
    wq_d = nc.dram_tensor("Wq", [DM, 3 * H * D], f32, kind="ExternalInput")
    bq_d = nc.dram_tensor("bq", [3 * H * D], f32, kind="ExternalInput")
    wk_d = nc.dram_tensor("Wk", [DM, 2 * H * D], f32, kind="ExternalInput")
    bk_d = nc.dram_tensor("bk", [2 * H * D], f32, kind="ExternalInput")
    wv_d = nc.dram_tensor("Wv", [DM, H * D], f32, kind="ExternalInput")
    bv_d = nc.dram_tensor("bv", [H * D], f32, kind="ExternalInput")
    gamma_d = nc.dram_tensor("gamma", [D], f32, kind="ExternalInput")
    beta_d = nc.dram_tensor("beta", [D], f32, kind="ExternalInput")
    lam_d = nc.dram_tensor("lam", [1], f32, kind="ExternalInput")
    li_d = nc.dram_tensor("lambda_init", [1], f32, kind="ExternalInput")
    out_d = nc.dram_tensor("out", [S, DM], f32, kind="ExternalOutput")

    ts_ = nc.vector.tensor_scalar
    stt = nc.vector.scalar_tensor_tensor

    with tile.TileContext(nc) as tc:
        with tc.tile_pool(name="consts", bufs=1) as consts, \
             tc.tile_pool(name="persist", bufs=1) as persist:

            # ---------- constants ----------
            ident = consts.tile([128, 128], f32, tag="ident", name="ident")
            make_identity(nc, ident)
            ident_b = consts.tile([128, 128], bf16, tag="ident_b", name="ident_b")
            make_identity(nc, ident_b)

            # block-diagonal group matrix: IND2[d', d] = 1 iff d'//8 == d//8
            ind2 = consts.tile([64, 64], f32, tag="ind2", name="ind2")
            nc.gpsimd.memset(ind2, 1.0)
            nc.gpsimd.affine_select(
                out=ind2, in_=ind2, compare_op=OP.is_ge, fill=0.0,
                base=0, pattern=[[-8, 8], [0, 8]], channel_multiplier=1)
            nc.gpsimd.affine_select(
                out=ind2, in_=ind2, compare_op=OP.is_ge, fill=0.0,
                base=7, pattern=[[8, 8], [0, 8]], channel_multiplier=-1)

            # selectors for the r-row broadcast matmul (per pair-half)
            # SP rows: [0]=sum1(even half), [1]=sum2, [2]=sum1(odd), [3]=sum2
            # sel[half][p, x] = 1 iff (x - 64p + 128*half) in [0, 64)
            sel = []
            for half in range(2):
                s_t = consts.tile([4, 128], f32, tag=f"sel{half}", name=f"sel{half}")
                nc.gpsimd.memset(s_t, 1.0)
                nc.gpsimd.affine_select(
                    out=s_t, in_=s_t, compare_op=OP.is_ge, fill=0.0,
                    base=128 * half, pattern=[[1, 128]], channel_multiplier=-64)
                nc.gpsimd.affine_select(
                    out=s_t, in_=s_t, compare_op=OP.is_ge, fill=0.0,
                    base=63 - 128 * half, pattern=[[-1, 128]], channel_multiplier=64)
                sel.append(s_t)

            # scalar columns
            lam64 = consts.tile([64, 1], f32, tag="lam64", name="lam64")
            nc.gpsimd.dma_start(out=lam64, in_=lam_d[:].to_broadcast([64, 1]))
            li64 = consts.tile([64, 1], f32, tag="li64", name="li64")
            nc.gpsimd.dma_start(out=li64, in_=li_d[:].to_broadcast([64, 1]))
            neglam64 = consts.tile([64, 1], f32, tag="neglam64", name="neglam64")
            ts_(neglam64, lam64, -1.0, None, OP.mult)
            onelam64 = consts.tile([64, 1], f32, tag="onelam64", name="onelam64")
            ts_(onelam64, lam64, -1.0, 1.0, OP.mult, OP.add)   # 1 - lam
            halfli = consts.tile([64, 1], f32, tag="halfli", name="halfli")
            ts_(halfli, li64, -0.5, 0.5, OP.mult, OP.add)      # 0.5*(1-li)

            gamma_c = consts.tile([64, 1], f32, tag="gamma_c", name="gamma_c")
            nc.sync.dma_start(out=gamma_c, in_=gamma_d[:])
            beta_c = consts.tile([64, 1], f32, tag="beta_c", name="beta_c")
            nc.sync.dma_start(out=beta_c, in_=beta_d[:])
            bb64 = consts.tile([64, 1], f32, tag="bb64", name="bb64")
            ts_(bb64, beta_c, halfli, None, OP.mult)           # beta*0.5*(1-li)

            # v-bias columns per head and C = bv*(1-lam) fold
            bvc = consts.tile([64, 8], f32, tag="bvc", name="bvc")
            nc.sync.dma_start(
                out=bvc, in_=bv_d[:].rearrange("(h d) -> d h", d=64))
            cc = consts.tile([64, 8], f32, tag="cc", name="cc")
            ts_(cc, bvc, onelam64, None, OP.mult)

            # bias columns: per-head stacked [q1|q2] / [k1|k2] are contiguous
            # 128-element runs of bq/bk; gate needs a gathered layout.
            bqp = consts.tile([128, 8], f32, tag="bqp", name="bqp")
            nc.sync.dma_start(
                out=bqp,
                in_=bq_d[:].rearrange("(h blk) -> blk h", blk=192)[0:128, :])
            bkp = consts.tile([128, 8], f32, tag="bkp", name="bkp")
            nc.sync.dma_start(
                out=bkp,
                in_=bk_d[:].rearrange("(h blk) -> blk h", blk=128))
            bg = consts.tile([128, 4], f32, tag="bg", name="bg")
            bqv = bq_d[:].rearrange("(h blk) -> h blk", blk=192)
            for p in range(4):
                nc.sync.dma_start(out=bg[:, p:p + 1],
                                  in_=bqv[2 * p:2 * p + 2, 128:192])

            # persistent projection outputs (bf16, d-major)
            # qp/kp[h]: rows 0-63 = q1/k1 of head h, rows 64-127 = q2/k2
            qp = [persist.tile([128, S], bf16, tag=f"qp{h}", name=f"qp{h}") for h in range(8)]
            # zero-padded key tiles: kz1[h] rows 0-63 = k1 (rest 0),
            # kz2[h] rows 64-127 = k2 (rest 0) -> K=128 score matmuls
            kz1 = [persist.tile([128, S], bf16, tag=f"kz1{h}", name=f"kz1{h}") for h in range(8)]
            kz2 = [persist.tile([128, S], bf16, tag=f"kz2{h}", name=f"kz2{h}") for h in range(8)]
            for h in range(8):
                nc.vector.memset(kz1[h][64:128, :], 0.0)
                nc.vector.memset(kz2[h][0:64, :], 0.0)
            # gate stays head-pair packed: gt[p] rows 0-63 = head 2p, 64-127 = 2p+1
            gt = [persist.tile([128, S], bf16, tag=f"gt{p}", name=f"gt{p}") for p in range(4)]
            va = [persist.tile([128, 8, 65], bf16, tag=f"va{i}", name=f"va{i}") for i in range(NJ)]
            ypair = [persist.tile([128, S], f32, tag=f"yp{p}", name=f"yp{p}") for p in range(4)]
            sumcol = persist.tile([64, 16], f32, tag="sumcol", name="sumcol")

            # ---------- phase 1: load + transpose inputs (DMA only) ----------
            # fp32 DRAM -> (cast DMA) -> bf16 DRAM scratch -> (xbar transpose
            # DMA) -> x^T bf16 in SBUF, 4 tiles of (128, S) per tensor.
            GRP = min(4, NJ)
            with tc.tile_pool(name="xin", bufs=3) as xin_pool, \
                 tc.tile_pool(name="xtp", bufs=1) as xtp, \
                 tc.tile_pool(name="wload", bufs=1) as wpool, \
                 tc.tile_pool(name="ps_in", bufs=1, space="PSUM") as ps_in, \
                 tc.tile_pool(name="ps_proj", bufs=4, space="PSUM") as ps_proj:

                def transpose_input(x_dram, nm):
                    xt = [xtp.tile([128, S], bf16, tag=f"xt{nm}{c}", name=f"xt{nm}{c}")
                          for c in range(4)]
                    tp_cur = [None] * 4
                    for i in range(NJ):
                        xs = xin_pool.tile([128, DM], f32, tag="xs", name="xs")
                        nc.sync.dma_start(out=xs, in_=x_dram[128 * i:128 * (i + 1), :])
                        xq = xin_pool.tile([128, DM], bf16, tag="xin", name="xin")
                        nc.vector.tensor_copy(xq, xs)
                        if i % GRP == 0:
                            for c in range(4):
                                tp_cur[c] = ps_in.tile(
                                    [128, 128 * GRP], bf16, tag=f"tp{c}", name=f"tp{c}")
                        for c in range(4):
                            nc.tensor.transpose(
                                tp_cur[c][:, 128 * (i % GRP):128 * (i % GRP + 1)],
                                xq[:, 128 * c:128 * (c + 1)], ident_b)
                        if i % GRP == GRP - 1:
                            base = 128 * GRP * (i // GRP)
                            for c in range(4):
                                nc.vector.tensor_copy(
                                    xt[c][:, base:base + 128 * GRP], tp_cur[c])
                    return xt

                # --- query path: qp[h] then gate ---
                # (x casts issue first so the SWDGE queue isn't stuck behind
                # the 6MB of weight casts at kernel start)
                xtq = transpose_input(q_d, "q")
                wqf = [wpool.tile([128, 3 * H * D], bf16, tag=f"wqf{r}", name=f"wqf{r}") for r in range(4)]
                wkf = [wpool.tile([128, 2 * H * D], bf16, tag=f"wkf{r}", name=f"wkf{r}") for r in range(4)]
                wvf = [wpool.tile([128, H * D], bf16, tag=f"wvf{r}", name=f"wvf{r}") for r in range(4)]
                # stage fp32 weights via HWDGE (fast, parallel to the x casts
                # on the SWDGE queue), downcast on the otherwise-idle ACT
                for r in range(4):
                    wsq = wpool.tile([128, 3 * H * D], f32, tag=f"wsq{r}", name=f"wsq{r}")
                    nc.sync.dma_start(out=wsq, in_=wq_d[128 * r:128 * (r + 1), :])
                    nc.scalar.copy(wqf[r], wsq)
                for r in range(4):
                    wsk = wpool.tile([128, 2 * H * D], f32, tag=f"wsk{r}", name=f"wsk{r}")
                    nc.sync.dma_start(out=wsk, in_=wk_d[128 * r:128 * (r + 1), :])
                    nc.scalar.copy(wkf[r], wsk)
                    wsv = wpool.tile([128, H * D], f32, tag=f"wsv{r}", name=f"wsv{r}")
                    nc.sync.dma_start(out=wsv, in_=wv_d[128 * r:128 * (r + 1), :])
                    nc.scalar.copy(wvf[r], wsv)
                for h in range(8):
                    for n in range(NN):
                        ps = ps_proj.tile([128, CH], f32, tag="proj", name="proj")
                        for r in range(4):
                            nc.tensor.matmul(
                                ps, wqf[r][:, 192 * h:192 * h + 128],
                                xtq[r][:, CH * n:CH * (n + 1)],
                                start=(r == 0), stop=(r == 3))
                        nc.scalar.activation(
                            qp[h][:, CH * n:CH * (n + 1)], ps, AF.Identity,
                            bias=bqp[:, h:h + 1])
                # gate: pre-gathered pair-packed weight tiles (the 64-col
                # blocks of heads 2p/2p+1 collected by the load DMA)
                wgt = []
                for r in range(4):
                    w_t = wpool.tile([128, 512], bf16, tag=f"wg{r}", name=f"wg{r}")
                    nc.gpsimd.dma_start(
                        out=w_t,
                        in_=wq_d[128 * r:128 * (r + 1), :].rearrange(
                            "k (h blk) -> k h blk", blk=192)[:, :, 128:192])
                    wgt.append(w_t)
                for p in range(4):
                    for n in range(NN):
                        ps = ps_proj.tile([128, CH], f32, tag="proj", name="proj")
                        for r in range(4):
                            nc.tensor.matmul(
                                ps, wgt[r][:, 128 * p:128 * (p + 1)],
                                xtq[r][:, CH * n:CH * (n + 1)],
                                start=(r == 0), stop=(r == 3))
                        nc.scalar.activation(
                            gt[p][:, CH * n:CH * (n + 1)], ps, AF.Identity,
                            bias=bg[:, p:p + 1])

                # --- key path ---
                xtk = transpose_input(k_d, "k")
                for h in range(8):
                    for n in range(NN):
                        ps = ps_proj.tile([128, CH], f32, tag="proj", name="proj")
                        for r in range(4):
                            nc.tensor.matmul(
                                ps, wkf[r][:, 128 * h:128 * (h + 1)],
                                xtk[r][:, CH * n:CH * (n + 1)],
                                start=(r == 0), stop=(r == 3))
                        nc.scalar.activation(
                            kz1[h][0:64, CH * n:CH * (n + 1)], ps[0:64, :],
                            AF.Identity, bias=bkp[0:64, h:h + 1])
                        nc.scalar.activation(
                            kz2[h][64:128, CH * n:CH * (n + 1)], ps[64:128, :],
                            AF.Identity, bias=bkp[64:128, h:h + 1])

                # --- values path (q-major, interleaved into v_aug + ones) ---
                xtv = transpose_input(v_d, "v")
                for i in range(NJ):
                    ps = ps_proj.tile([128, 512], f32, tag="proj", name="proj")
                    for r in range(4):
                        nc.tensor.matmul(
                            ps, xtv[r][:, 128 * i:128 * (i + 1)], wvf[r],
                            start=(r == 0), stop=(r == 3))
                    nc.scalar.copy(
                        va[i][:, :, 0:64],
                        ps.rearrange("p (h d) -> p h d", d=64))
                    nc.gpsimd.memset(va[i][:, :, 64:65], 1.0)

                # gate tanh now (ACT is free here; result only needed at the
                # very end) -- th_t lives in the persist pool
                th_t = [persist.tile([128, S], f32, tag=f"th{p}", name=f"th{p}")
                        for p in range(4)]
                for p in range(4):
                    nc.scalar.activation(th_t[p], gt[p], AF.Tanh, scale=0.5)

            # ---------- phase 2: attention per head (pairs for epilogue) ----
            with tc.tile_pool(name="ps_att", bufs=1, space="PSUM") as ps_att, \
                 tc.tile_pool(name="ps_o", bufs=1, space="PSUM") as ps_o, \
                 tc.tile_pool(name="expp", bufs=2) as expp, \
                 tc.tile_pool(name="osp", bufs=2) as osp, \
                 tc.tile_pool(name="spp", bufs=2) as spp:

                def emit_combine(sp, os_t, p):
                    rp = spp.tile([4, S], f32, tag="rp", name="rp", bufs=1)
                    rscr = spp.tile([4, S], f32, tag="rscr", name="rscr", bufs=1)
                    nc.vector.reciprocal_approx_accurate(rp, sp, rscr)
                    rst = []
                    for i in range(4):
                        r_t = spp.tile([1, S], f32, tag=f"rst{i}", name=f"rst{i}", bufs=1)
                        nc.sync.dma_start(out=r_t, in_=rp[i:i + 1, :])
                        rst.append(r_t)
                    for half in range(2):
                        h = 2 * p + half
                        os1, os2 = os_t[half]
                        bcs1 = spp.tile([64, S], f32, tag="bcs1", name="bcs1", bufs=1)
                        bcs2 = spp.tile([64, S], f32, tag="bcs2", name="bcs2", bufs=1)
                        nc.gpsimd.partition_broadcast(
                            bcs1, rst[2 * half][0:1, :], channels=64)
                        nc.gpsimd.partition_broadcast(
                            bcs2, rst[2 * half + 1][0:1, :], channels=64)
                        nc.vector.tensor_mul(os1[0:64, :], os1[0:64, :], bcs1)
                        stt(os2[0:64, :], os2[0:64, :], neglam64, bcs2,
                            OP.mult, OP.mult)
                        ydst = ypair[p][64 * half:64 * half + 64, :]
                        stt(ydst, os1[0:64, :], 1.0, os2[0:64, :],
                            OP.bypass, OP.add, accum_out=sumcol[:, h:h + 1])
                        stt(os1[0:64, :], ydst, 1.0, ydst, OP.mult, OP.mult,
                            accum_out=sumcol[:, 8 + h:9 + h])

                for p in range(4):
                    os_t = {}
                    sp = spp.tile([4, S], f32, tag="sp", name="sp")
                    for half in range(2):
                        h = 2 * p + half
                        o_ps = {}
                        for t in (1, 2):
                            o_ps[t] = ps_o.tile([65, S], f32, tag=f"o{t}", name=f"o{t}")
                        for j in range(NJ):
                            for t, kz_ in ((1, kz1), (2, kz2)):
                                s_ps = ps_att.tile([128, S], f32, tag=f"s{t}", name=f"s{t}")
                                for n in range(NN):
                                    nc.tensor.matmul(
                                        s_ps[:, CH * n:CH * (n + 1)],
                                        kz_[h][:, 128 * j:128 * (j + 1)],
                                        qp[h][:, CH * n:CH * (n + 1)],
                                        start=True, stop=True)
                                ex = expp.tile([128, S], bf16, tag=f"exp{t}", name=f"exp{t}")
                                nc.scalar.activation(ex, s_ps, AF.Exp, scale=INV)
                                for n in range(NN):
                                    nc.tensor.matmul(
                                        o_ps[t][:, CH * n:CH * (n + 1)],
                                        va[j][:, h, :],
                                        ex[:, CH * n:CH * (n + 1)],
                                        start=(j == 0), stop=(j == NJ - 1))
                        os1 = osp.tile([65, S], f32, tag=f"os1_{half}", name=f"os1_{half}")
                        os2 = osp.tile([65, S], f32, tag=f"os2_{half}", name=f"os2_{half}")
                        nc.vector.tensor_copy(os1, o_ps[1])
                        nc.vector.tensor_copy(os2, o_ps[2])
                        os_t[half] = (os1, os2)
                        nc.sync.dma_start(out=sp[2 * half:2 * half + 1, :],
                                          in_=os1[64:65, :])
                        nc.sync.dma_start(out=sp[2 * half + 1:2 * half + 2, :],
                                          in_=os2[64:65, :])
                    emit_combine(sp, os_t, p)

            # ---------- phase 3: stats, groupnorm, gate, output ----------
            with tc.tile_pool(name="tailp", bufs=1) as tailp, \
                 tc.tile_pool(name="oq", bufs=3) as oqp, \
                 tc.tile_pool(name="ps_tail", bufs=2, space="PSUM") as ps_tail:

                tot = tailp.tile([64, 2], f32, tag="tot", name="tot")
                nc.vector.tensor_reduce(
                    tot, sumcol.rearrange("p (t h) -> p t h", h=8),
                    axis=AX.X, op=OP.add)
                # bias-C (bv) corrections to the raw-Y stats
                csc = tailp.tile([64, 8], f32, tag="csc", name="csc")
                nc.vector.tensor_mul(csc, cc, sumcol[:, 0:8])
                cy64 = tailp.tile([64, 1], f32, tag="cy64", name="cy64")
                nc.vector.tensor_reduce(cy64, csc, axis=AX.X, op=OP.add)
                nc.vector.tensor_mul(csc, cc, cc)
                csq64 = tailp.tile([64, 1], f32, tag="csq64", name="csq64")
                nc.vector.tensor_reduce(csq64, csc, axis=AX.X, op=OP.add)
                csum64 = tailp.tile([64, 1], f32, tag="csum64", name="csum64")
                nc.vector.tensor_reduce(csum64, cc, axis=AX.X, op=OP.add)
                tot2 = tailp.tile([64, 2], f32, tag="tot2", name="tot2")
                stt(tot2[:, 0:1], csum64, float(S), tot[:, 0:1], OP.mult, OP.add)
                stt(tot2[:, 1:2], cy64, 2.0, tot[:, 1:2], OP.mult, OP.add)
                stt(tot2[:, 1:2], csq64, float(S), tot2[:, 1:2], OP.mult, OP.add)

                ms_ps = ps_tail.tile([64, 2], f32, tag="ms", name="ms")
                nc.tensor.matmul(ms_ps, ind2, tot2, start=True, stop=True)
                mean64 = tailp.tile([64, 1], f32, tag="mean64", name="mean64")
                ts_(mean64, ms_ps[:, 0:1], 1.0 / CNT, None, OP.mult)
                e264 = tailp.tile([64, 1], f32, tag="e264", name="e264")
                ts_(e264, ms_ps[:, 1:2], 1.0 / CNT, None, OP.mult)
                nm2 = tailp.tile([64, 1], f32, tag="nm2", name="nm2")
                ts_(nm2, mean64, mean64, -1.0, OP.mult, OP.mult)
                veps = tailp.tile([64, 1], f32, tag="veps", name="veps")
                stt(veps, nm2, EPS, e264, OP.add, OP.add)
                sd = tailp.tile([64, 1], f32, tag="sd", name="sd")
                nc.scalar.activation(sd, veps, AF.Sqrt)
                rsd = tailp.tile([64, 1], f32, tag="rsd", name="rsd")
                nc.vector.reciprocal(rsd, sd)
                # one Newton step for rsqrt accuracy (ACT sqrt is loose)
                rr = tailp.tile([64, 1], f32, tag="rr", name="rr")
                nc.vector.tensor_mul(rr, rsd, rsd)
                nc.vector.tensor_mul(rr, rr, veps)
                ts_(rr, rr, -0.5, 1.5, OP.mult, OP.add)
                rstd = tailp.tile([64, 1], f32, tag="rstd", name="rstd")
                nc.vector.tensor_mul(rstd, rsd, rr)

                a64 = tailp.tile([64, 1], f32, tag="a64", name="a64")
                ts_(a64, rstd, gamma_c, halfli, OP.mult, OP.mult)
                cm = tailp.tile([64, 8], f32, tag="cm", name="cm")
                ts_(cm, cc, mean64, None, OP.subtract)
                ball = tailp.tile([64, 8], f32, tag="ball", name="ball")
                ts_(ball, cm, a64, bb64, OP.mult, OP.add)

                for p in range(4):
                    for half in range(2):
                        h = 2 * p + half
                        rows = ypair[p][64 * half:64 * half + 64, :]
                        ts_(rows, rows, a64, ball[:, h:h + 1], OP.mult, OP.add)
                    stt(ypair[p], th_t[p], 1.0, ypair[p], OP.add, OP.mult)

                for c in range(NJ):
                    tp_o = ps_tail.tile([128, 512], f32, tag="tp_out", name="tp_out")
                    for p in range(4):
                        nc.tensor.transpose(
                            tp_o[:, 128 * p:128 * (p + 1)],
                            ypair[p][:, 128 * c:128 * (c + 1)], ident)
                    oq = oqp.tile([128, 512], f32, tag="oq", name="oq")
                    nc.vector.tensor_copy(oq, tp_o)
                    nc.sync.dma_start(out=out_d[128 * c:128 * (c + 1), :], in_=oq)

    nc.finalize()
    return nc


_CACHE = {}


def _get_nc():
    if "nc" not in _CACHE:
        _CACHE["nc"] = build_nc(S_FULL)
    return _CACHE["nc"]


def run(inputs, trace=False, tmpdir=None):
    from concourse.bass_utils import run_bass_kernel_spmd
    nc = _get_nc()
    arrs = {k: np.asarray(v, dtype=np.float32) for k, v in inputs.items()}
    shared = {k: np.ascontiguousarray(arrs[k]) for k in
              ("Wq", "bq", "Wk", "bk", "Wv", "bv", "gamma", "beta",
               "lam", "lambda_init")}
    in_maps = []
    for i in range(B):
        m = dict(shared)
        m["query"] = np.ascontiguousarray(arrs["query"][i])
        m["key"] = np.ascontiguousarray(arrs["key"][i])
        m["values"] = np.ascontiguousarray(arrs["values"][i])
        in_maps.append(m)
    res = run_bass_kernel_spmd(nc, in_maps, core_ids=list(range(B)),
                               trace=trace, tmpdir=tmpdir)
    out = np.stack([res.results[i]["out"] for i in range(B)], axis=0)
    return out.astype(np.float32), res


def kernel(**inputs):
    out, _ = run(inputs)
    return out


# revision 23
# speedup vs baseline: 1.1824x; 1.0168x over previous
# Differential multi-head attention (dual softmax + GroupNorm + sigmoid gating)
# for Trainium2, batch-parallel across 8 NeuronCores (one batch row per core).
#
# Per-core math (batch b):
#   q = query @ Wq + bq -> per head: q1, q2, gate (each S x 64)
#   k = key   @ Wk + bk -> per head: k1, k2
#   v = values@ Wv + bv -> per head: v (S x 64)
#   attn = softmax(q1 k1^T / 8) - lam * softmax(q2 k2^T / 8)
#   out  = GroupNorm_{8 groups over d, reduced over (S, heads, d-in-group)}(attn @ v)
#   out  = out * (1 - lambda_init) * sigmoid(gate)
#
# Layout strategy: d-major ("transposed") attention: scores are computed as
# s^T (k on partitions, q free) so the attn@v contraction runs at K=128, and
# exp row-sums come free via a ones-column appended to v (M=65).  q1/q2 (and
# k1/k2) of each head live in complementary 64-partition halves of one tile,
# so the two K=64 score matmuls of a head occupy disjoint PE row-groups and
# can run concurrently.  Matmul inputs are bf16 (single-pass PE); accumulation,
# softmax normalization, GroupNorm and the final output stay fp32.
# sigmoid(x) = (tanh(x/2)+1)/2 keeps ACT in one table set (exp/tanh/square).

import numpy as np

B, S_FULL, H, D = 8, 1024, 8, 64
DM = H * D  # 512


def build_nc(S=1024):
    import concourse.bacc as bacc
    import concourse.bass as bass
    import concourse.tile as tile
    from concourse import mybir
    from concourse.masks import make_identity

    f32 = mybir.dt.float32
    bf16 = mybir.dt.bfloat16
    AF = mybir.ActivationFunctionType
    OP = mybir.AluOpType
    AX = mybir.AxisListType

    NJ = S // 128          # k/seq 128-tiles
    CH = min(512, S)       # fp32-out matmul chunk
    NN = max(1, S // CH)
    CNT = float(S * H * (D // H))  # groupnorm reduction count per group
    EPS = 1e-3
    INV = 0.125            # 1/sqrt(64)

    nc = bacc.Bacc(target_bir_lowering=False)
    q_d = nc.dram_tensor("query", [S, DM], f32, kind="ExternalInput")
    k_d = nc.dram_tensor("key", [S, DM], f32, kind="ExternalInput")
    v_d = nc.dram_tensor("values", [S, DM], f32, kind="ExternalInput")
    wq_d = nc.dram_tensor("Wq", [DM, 3 * H * D], f32, kind="ExternalInput")
    bq_d = nc.dram_tensor("bq", [3 * H * D], f32, kind="ExternalInput")
    wk_d = nc.dram_tensor("Wk", [DM, 2 * H * D], f32, kind="ExternalInput")
    bk_d = nc.dram_tensor("bk", [2 * H * D], f32, kind="ExternalInput")
    wv_d = nc.dram_tensor("Wv", [DM, H * D], f32, kind="ExternalInput")
    bv_d = nc.dram_tensor("bv", [H * D], f32, kind="ExternalInput")
    gamma_d = nc.dram_tensor("gamma", [D], f32, kind="ExternalInput")
    beta_d = nc.dram_tensor("beta", [D], f32, kind="ExternalInput")
    lam_d = nc.dram_tensor("lam", [1], f32, kind="ExternalInput")
    li_d = nc.dram_tensor("lambda_init", [1], f32, kind="ExternalInput")
    out_d = nc.dram_tensor("out", [S, DM], f32, kind="ExternalOutput")

    ts_ = nc.vector.tensor_scalar
    stt = nc.vector.scalar_tensor_tensor

    with tile.TileContext(nc) as tc:
        with tc.tile_pool(name="consts", bufs=1) as consts, \
             tc.tile_pool(name="persist", bufs=1) as persist:

            # ---------- constants ----------
            ident = consts.tile([128, 128], f32, tag="ident", name="ident")
            make_identity(nc, ident)
            ident_b = consts.tile([128, 128], bf16, tag="ident_b", name="ident_b")
            make_identity(nc, ident_b)

            # block-diagonal group matrix: IND2[d', d] = 1 iff d'//8 == d//8
            ind2 = consts.tile([64, 64], f32, tag="ind2", name="ind2")
            nc.gpsimd.memset(ind2, 1.0)
            nc.gpsimd.affine_select(
                out=ind2, in_=ind2, compare_op=OP.is_ge, fill=0.0,
                base=0, pattern=[[-8, 8], [0, 8]], channel_multiplier=1)
            nc.gpsimd.affine_select(
                out=ind2, in_=ind2, compare_op=OP.is_ge, fill=0.0,
                base=7, pattern=[[8, 8], [0, 8]], channel_multiplier=-1)

            # selectors for the r-row broadcast matmul (per pair-half)
            # SP rows: [0]=sum1(even half), [1]=sum2, [2]=sum1(odd), [3]=sum2
            # sel[half][p, x] = 1 iff (x - 64p + 128*half) in [0, 64)
            sel = []
            for half in range(2):
                s_t = consts.tile([4, 128], f32, tag=f"sel{half}", name=f"sel{half}")
                nc.gpsimd.memset(s_t, 1.0)
                nc.gpsimd.affine_select(
                    out=s_t, in_=s_t, compare_op=OP.is_ge, fill=0.0,
                    base=128 * half, pattern=[[1, 128]], channel_multiplier=-64)
                nc.gpsimd.affine_select(
                    out=s_t, in_=s_t, compare_op=OP.is_ge, fill=0.0,
                    base=63 - 128 * half, pattern=[[-1, 128]], channel_multiplier=64)
                sel.append(s_t)

            # scalar columns
            lam64 = consts.tile([64, 1], f32, tag="lam64", name="lam64")
            nc.gpsimd.dma_start(out=lam64, in_=lam_d[:].to_broadcast([64, 1]))
            li64 = consts.tile([64, 1], f32, tag="li64", name="li64")
            nc.gpsimd.dma_start(out=li64, in_=li_d[:].to_broadcast([64, 1]))
            neglam64 = consts.tile([64, 1], f32, tag="neglam64", name="neglam64")
            ts_(neglam64, lam64, -1.0, None, OP.mult)
            onelam64 = consts.tile([64, 1], f32, tag="onelam64", name="onelam64")
            ts_(onelam64, lam64, -1.0, 1.0, OP.mult, OP.add)   # 1 - lam
            halfli = consts.tile([64, 1], f32, tag="halfli", name="halfli")
            ts_(halfli, li64, -0.5, 0.5, OP.mult, OP.add)      # 0.5*(1-li)

            gamma_c = consts.tile([64, 1], f32, tag="gamma_c", name="gamma_c")
            nc.sync.dma_start(out=gamma_c, in_=gamma_d[:])
            beta_c = consts.tile([64, 1], f32, tag="beta_c", name="beta_c")
            nc.sync.dma_start(out=beta_c, in_=beta_d[:])
            bb64 = consts.tile([64, 1], f32, tag="bb64", name="bb64")
            ts_(bb64, beta_c, halfli, None, OP.mult)           # beta*0.5*(1-li)

            # v-bias columns per head and C = bv*(1-lam) fold
            bvc = consts.tile([64, 8], f32, tag="bvc", name="bvc")
            nc.sync.dma_start(
                out=bvc, in_=bv_d[:].rearrange("(h d) -> d h", d=64))
            cc = consts.tile([64, 8], f32, tag="cc", name="cc")
            ts_(cc, bvc, onelam64, None, OP.mult)

            # bias columns: per-head stacked [q1|q2] / [k1|k2] are contiguous
            # 128-element runs of bq/bk; gate needs a gathered layout.
            bqp = consts.tile([128, 8], f32, tag="bqp", name="bqp")
            nc.sync.dma_start(
                out=bqp,
                in_=bq_d[:].rearrange("(h blk) -> blk h", blk=192)[0:128, :])
            bkp = consts.tile([128, 8], f32, tag="bkp", name="bkp")
            nc.sync.dma_start(
                out=bkp,
                in_=bk_d[:].rearrange("(h blk) -> blk h", blk=128))
            bg = consts.tile([128, 4], f32, tag="bg", name="bg")
            bqv = bq_d[:].rearrange("(h blk) -> h blk", blk=192)
            for p in range(4):
                nc.sync.dma_start(out=bg[:, p:p + 1],
                                  in_=bqv[2 * p:2 * p + 2, 128:192])

            # persistent projection outputs (bf16, d-major)
            # qp/kp[h]: rows 0-63 = q1/k1 of head h, rows 64-127 = q2/k2
            qp = [persist.tile([128, S], bf16, tag=f"qp{h}", name=f"qp{h}") for h in range(8)]
            # zero-padded key tiles: kz1[h] rows 0-63 = k1 (rest 0),
            # kz2[h] rows 64-127 = k2 (rest 0) -> K=128 score matmuls
            kz1 = [persist.tile([128, S], bf16, tag=f"kz1{h}", name=f"kz1{h}") for h in range(8)]
            kz2 = [persist.tile([128, S], bf16, tag=f"kz2{h}", name=f"kz2{h}") for h in range(8)]
            for h in range(8):
                nc.vector.memset(kz1[h][64:128, :], 0.0)
                nc.vector.memset(kz2[h][0:64, :], 0.0)
            # gate stays head-pair packed: gt[p] rows 0-63 = head 2p, 64-127 = 2p+1
            gt = [persist.tile([128, S], bf16, tag=f"gt{p}", name=f"gt{p}") for p in range(4)]
            va = [persist.tile([128, 8, 65], bf16, tag=f"va{i}", name=f"va{i}") for i in range(NJ)]
            ypair = [persist.tile([128, S], f32, tag=f"yp{p}", name=f"yp{p}") for p in range(4)]
            sumcol = persist.tile([64, 16], f32, tag="sumcol", name="sumcol")

            # ---------- phase 1: load + transpose inputs (DMA only) ----------
            # fp32 DRAM -> (cast DMA) -> bf16 DRAM scratch -> (xbar transpose
            # DMA) -> x^T bf16 in SBUF, 4 tiles of (128, S) per tensor.
            GRP = min(4, NJ)
            with tc.tile_pool(name="xin", bufs=3) as xin_pool, \
                 tc.tile_pool(name="xtp", bufs=1) as xtp, \
                 tc.tile_pool(name="wload", bufs=1) as wpool, \
                 tc.tile_pool(name="ps_in", bufs=1, space="PSUM") as ps_in, \
                 tc.tile_pool(name="ps_proj", bufs=4, space="PSUM") as ps_proj:

                def transpose_input(x_dram, nm):
                    xt = [xtp.tile([128, S], bf16, tag=f"xt{nm}{c}", name=f"xt{nm}{c}")
                          for c in range(4)]
                    tp_cur = [None] * 4
                    for i in range(NJ):
                        xs = xin_pool.tile([128, DM], f32, tag="xs", name="xs")
                        nc.sync.dma_start(out=xs, in_=x_dram[128 * i:128 * (i + 1), :])
                        xq = xin_pool.tile([128, DM], bf16, tag="xin", name="xin")
                        nc.vector.tensor_copy(xq, xs)
                        if i % GRP == 0:
                            for c in range(4):
                                tp_cur[c] = ps_in.tile(
                                    [128, 128 * GRP], bf16, tag=f"tp{c}", name=f"tp{c}")
                        for c in range(4):
                            nc.tensor.transpose(
                                tp_cur[c][:, 128 * (i % GRP):128 * (i % GRP + 1)],
                                xq[:, 128 * c:128 * (c + 1)], ident_b)
                        if i % GRP == GRP - 1:
                            base = 128 * GRP * (i // GRP)
                            for c in range(4):
                                nc.vector.tensor_copy(
                                    xt[c][:, base:base + 128 * GRP], tp_cur[c])
                    return xt

                # --- query path: qp[h] then gate ---
                # (x casts issue first so the SWDGE queue isn't stuck behind
                # the 6MB of weight casts at kernel start)
                xtq = transpose_input(q_d, "q")
                wqf = [wpool.tile([128, 3 * H * D], bf16, tag=f"wqf{r}", name=f"wqf{r}") for r in range(4)]
                wkf = [wpool.tile([128, 2 * H * D], bf16, tag=f"wkf{r}", name=f"wkf{r}") for r in range(4)]
                wvf = [wpool.tile([128, H * D], bf16, tag=f"wvf{r}", name=f"wvf{r}") for r in range(4)]
                # stage fp32 weights via HWDGE (fast, parallel to the x casts
                # on the SWDGE queue), downcast on the otherwise-idle ACT
                for r in range(4):
                    wsq = wpool.tile([128, 3 * H * D], f32, tag=f"wsq{r}", name=f"wsq{r}")
                    nc.sync.dma_start(out=wsq, in_=wq_d[128 * r:128 * (r + 1), :])
                    nc.scalar.copy(wqf[r], wsq)
                for r in range(4):
                    wsk = wpool.tile([128, 2 * H * D], f32, tag=f"wsk{r}", name=f"wsk{r}")
                    nc.sync.dma_start(out=wsk, in_=wk_d[128 * r:128 * (r + 1), :])
                    nc.scalar.copy(wkf[r], wsk)
                    wsv = wpool.tile([128, H * D], f32, tag=f"wsv{r}", name=f"wsv{r}")
                    nc.sync.dma_start(out=wsv, in_=wv_d[128 * r:128 * (r + 1), :])
                    nc.scalar.copy(wvf[r], wsv)
                for h in range(8):
                    for n in range(NN):
                        ps = ps_proj.tile([128, CH], f32, tag="proj", name="proj")
                        for r in range(4):
                            nc.tensor.matmul(
                                ps, wqf[r][:, 192 * h:192 * h + 128],
                                xtq[r][:, CH * n:CH * (n + 1)],
                                start=(r == 0), stop=(r == 3))
                        nc.scalar.activation(
                            qp[h][:, CH * n:CH * (n + 1)], ps, AF.Identity,
                            bias=bqp[:, h:h + 1])
                # gate: pre-gathered pair-packed weight tiles (the 64-col
                # blocks of heads 2p/2p+1 collected by the load DMA)
                wgt = []
                for r in range(4):
                    w_t = wpool.tile([128, 512], bf16, tag=f"wg{r}", name=f"wg{r}")
                    nc.gpsimd.dma_start(
                        out=w_t,
                        in_=wq_d[128 * r:128 * (r + 1), :].rearrange(
                            "k (h blk) -> k h blk", blk=192)[:, :, 128:192])
                    wgt.append(w_t)
                for p in range(4):
                    for n in range(NN):
                        ps = ps_proj.tile([128, CH], f32, tag="proj", name="proj")
                        for r in range(4):
                            nc.tensor.matmul(
                                ps, wgt[r][:, 128 * p:128 * (p + 1)],
                                xtq[r][:, CH * n:CH * (n + 1)],
                                start=(r == 0), stop=(r == 3))
                        nc.scalar.activation(
                            gt[p][:, CH * n:CH * (n + 1)], ps, AF.Identity,
                            bias=bg[:, p:p + 1])

                # --- key path ---
                xtk = transpose_input(k_d, "k")
                for h in range(8):
                    for n in range(NN):
                        ps = ps_proj.tile([128, CH], f32, tag="proj", name="proj")
                        for r in range(4):
                            nc.tensor.matmul(
                                ps, wkf[r][:, 128 * h:128 * (h + 1)],
                                xtk[r][:, CH * n:CH * (n + 1)],
                                start=(r == 0), stop=(r == 3))
                        nc.scalar.activation(
                            kz1[h][0:64, CH * n:CH * (n + 1)], ps[0:64, :],
                            AF.Identity, bias=bkp[0:64, h:h + 1])
                        nc.scalar.activation(
                            kz2[h][64:128, CH * n:CH * (n + 1)], ps[64:128, :],
                            AF.Identity, bias=bkp[64:128, h:h + 1])

                # --- values path (q-major, interleaved into v_aug + ones) ---
                xtv = transpose_input(v_d, "v")
                for i in range(NJ):
                    ps = ps_proj.tile([128, 512], f32, tag="proj", name="proj")
                    for r in range(4):
                        nc.tensor.matmul(
                            ps, xtv[r][:, 128 * i:128 * (i + 1)], wvf[r],
                            start=(r == 0), stop=(r == 3))
                    nc.scalar.copy(
                        va[i][:, :, 0:64],
                        ps.rearrange("p (h d) -> p h d", d=64))
                    nc.gpsimd.memset(va[i][:, :, 64:65], 1.0)

                # gate tanh now (ACT is free here; result only needed at the
                # very end) -- th_t lives in the persist pool
                th_t = [persist.tile([128, S], f32, tag=f"th{p}", name=f"th{p}")
                        for p in range(4)]
                for p in range(4):
                    nc.scalar.activation(th_t[p], gt[p], AF.Tanh, scale=0.5)

            # ---------- phase 2: attention per head (pairs for epilogue) ----
            with tc.tile_pool(name="ps_att", bufs=1, space="PSUM") as ps_att, \
                 tc.tile_pool(name="ps_o", bufs=1, space="PSUM") as ps_o, \
                 tc.tile_pool(name="expp", bufs=3) as expp, \
                 tc.tile_pool(name="osp", bufs=2) as osp, \
                 tc.tile_pool(name="spp", bufs=2) as spp:

                def emit_combine(sp, os_t, p):
                    rp = spp.tile([4, S], f32, tag="rp", name="rp", bufs=1)
                    rscr = spp.tile([4, S], f32, tag="rscr", name="rscr", bufs=1)
                    nc.vector.reciprocal_approx_accurate(rp, sp, rscr)
                    rst = []
                    for i in range(4):
                        r_t = spp.tile([1, S], f32, tag=f"rst{i}", name=f"rst{i}", bufs=1)
                        nc.sync.dma_start(out=r_t, in_=rp[i:i + 1, :])
                        rst.append(r_t)
                    for half in range(2):
                        h = 2 * p + half
                        os1, os2 = os_t[half]
                        bcs1 = spp.tile([64, S], f32, tag="bcs1", name="bcs1", bufs=1)
                        bcs2 = spp.tile([64, S], f32, tag="bcs2", name="bcs2", bufs=1)
                        nc.gpsimd.partition_broadcast(
                            bcs1, rst[2 * half][0:1, :], channels=64)
                        nc.gpsimd.partition_broadcast(
                            bcs2, rst[2 * half + 1][0:1, :], channels=64)
                        nc.vector.tensor_mul(os1[0:64, :], os1[0:64, :], bcs1)
                        stt(os2[0:64, :], os2[0:64, :], neglam64, bcs2,
                            OP.mult, OP.mult)
                        ydst = ypair[p][64 * half:64 * half + 64, :]
                        stt(ydst, os1[0:64, :], 1.0, os2[0:64, :],
                            OP.bypass, OP.add, accum_out=sumcol[:, h:h + 1])
                        stt(os1[0:64, :], ydst, 1.0, ydst, OP.mult, OP.mult,
                            accum_out=sumcol[:, 8 + h:9 + h])

                for p in range(4):
                    os_t = {}
                    sp = spp.tile([4, S], f32, tag="sp", name="sp")
                    for half in range(2):
                        h = 2 * p + half
                        o_ps = {}
                        for t in (1, 2):
                            o_ps[t] = ps_o.tile([65, S], f32, tag=f"o{t}", name=f"o{t}")
                        for j in range(NJ):
                            for t, kz_ in ((1, kz1), (2, kz2)):
                                s_ps = ps_att.tile([128, S], f32, tag=f"s{t}", name=f"s{t}")
                                for n in range(NN):
                                    nc.tensor.matmul(
                                        s_ps[:, CH * n:CH * (n + 1)],
                                        kz_[h][:, 128 * j:128 * (j + 1)],
                                        qp[h][:, CH * n:CH * (n + 1)],
                                        start=True, stop=True)
                                ex = expp.tile([128, S], bf16, tag=f"exp{t}", name=f"exp{t}")
                                nc.scalar.activation(ex, s_ps, AF.Exp, scale=INV)
                                for n in range(NN):
                                    nc.tensor.matmul(
                                        o_ps[t][:, CH * n:CH * (n + 1)],
                                        va[j][:, h, :],
                                        ex[:, CH * n:CH * (n + 1)],
                                        start=(j == 0), stop=(j == NJ - 1))
                        os1 = osp.tile([65, S], f32, tag=f"os1_{half}", name=f"os1_{half}")
                        os2 = osp.tile([65, S], f32, tag=f"os2_{half}", name=f"os2_{half}")
                        nc.vector.tensor_copy(os1, o_ps[1])
                        nc.vector.tensor_copy(os2, o_ps[2])
                        os_t[half] = (os1, os2)
                        nc.sync.dma_start(out=sp[2 * half:2 * half + 1, :],
                                          in_=os1[64:65, :])
                        nc.sync.dma_start(out=sp[2 * half + 1:2 * half + 2, :],
                                          in_=os2[64:65, :])
                    emit_combine(sp, os_t, p)

            # ---------- phase 3: stats, groupnorm, gate, output ----------
            with tc.tile_pool(name="tailp", bufs=1) as tailp, \
                 tc.tile_pool(name="oq", bufs=3) as oqp, \
                 tc.tile_pool(name="ps_tail", bufs=2, space="PSUM") as ps_tail:

                tot = tailp.tile([64, 2], f32, tag="tot", name="tot")
                nc.vector.tensor_reduce(
                    tot, sumcol.rearrange("p (t h) -> p t h", h=8),
                    axis=AX.X, op=OP.add)
                # bias-C (bv) corrections to the raw-Y stats
                csc = tailp.tile([64, 8], f32, tag="csc", name="csc")
                nc.vector.tensor_mul(csc, cc, sumcol[:, 0:8])
                cy64 = tailp.tile([64, 1], f32, tag="cy64", name="cy64")
                nc.vector.tensor_reduce(cy64, csc, axis=AX.X, op=OP.add)
                nc.vector.tensor_mul(csc, cc, cc)
                csq64 = tailp.tile([64, 1], f32, tag="csq64", name="csq64")
                nc.vector.tensor_reduce(csq64, csc, axis=AX.X, op=OP.add)
                csum64 = tailp.tile([64, 1], f32, tag="csum64", name="csum64")
                nc.vector.tensor_reduce(csum64, cc, axis=AX.X, op=OP.add)
                tot2 = tailp.tile([64, 2], f32, tag="tot2", name="tot2")
                stt(tot2[:, 0:1], csum64, float(S), tot[:, 0:1], OP.mult, OP.add)
                stt(tot2[:, 1:2], cy64, 2.0, tot[:, 1:2], OP.mult, OP.add)
                stt(tot2[:, 1:2], csq64, float(S), tot2[:, 1:2], OP.mult, OP.add)

                ms_ps = ps_tail.tile([64, 2], f32, tag="ms", name="ms")
                nc.tensor.matmul(ms_ps, ind2, tot2, start=True, stop=True)
                mean64 = tailp.tile([64, 1], f32, tag="mean64", name="mean64")
                ts_(mean64, ms_ps[:, 0:1], 1.0 / CNT, None, OP.mult)
                e264 = tailp.tile([64, 1], f32, tag="e264", name="e264")
                ts_(e264, ms_ps[:, 1:2], 1.0 / CNT, None, OP.mult)
                nm2 = tailp.tile([64, 1], f32, tag="nm2", name="nm2")
                ts_(nm2, mean64, mean64, -1.0, OP.mult, OP.mult)
                veps = tailp.tile([64, 1], f32, tag="veps", name="veps")
                stt(veps, nm2, EPS, e264, OP.add, OP.add)
                sd = tailp.tile([64, 1], f32, tag="sd", name="sd")
                nc.scalar.activation(sd, veps, AF.Sqrt)
                rsd = tailp.tile([64, 1], f32, tag="rsd", name="rsd")
                nc.vector.reciprocal(rsd, sd)
                # one Newton step for rsqrt accuracy (ACT sqrt is loose)
                rr = tailp.tile([64, 1], f32, tag="rr", name="rr")
                nc.vector.tensor_mul(rr, rsd, rsd)
                nc.vector.tensor_mul(rr, rr, veps)
                ts_(rr, rr, -0.5, 1.5, OP.mult, OP.add)
                rstd = tailp.tile([64, 1], f32, tag="rstd", name="rstd")
                nc.vector.tensor_mul(rstd, rsd, rr)

                a64 = tailp.tile([64, 1], f32, tag="a64", name="a64")
                ts_(a64, rstd, gamma_c, halfli, OP.mult, OP.mult)
                cm = tailp.tile([64, 8], f32, tag="cm", name="cm")
                ts_(cm, cc, mean64, None, OP.subtract)
                ball = tailp.tile([64, 8], f32, tag="ball", name="ball")
                ts_(ball, cm, a64, bb64, OP.mult, OP.add)

                for p in range(4):
                    for half in range(2):
                        h = 2 * p + half
                        rows = ypair[p][64 * half:64 * half + 64, :]
                        ts_(rows, rows, a64, ball[:, h:h + 1], OP.mult, OP.add)
                    stt(ypair[p], th_t[p], 1.0, ypair[p], OP.add, OP.mult)

                for c in range(NJ):
                    tp_o = ps_tail.tile([128, 512], f32, tag="tp_out", name="tp_out")
                    for p in range(4):
                        nc.tensor.transpose(
                            tp_o[:, 128 * p:128 * (p + 1)],
                            ypair[p][:, 128 * c:128 * (c + 1)], ident)
                    oq = oqp.tile([128, 512], f32, tag="oq", name="oq")
                    nc.vector.tensor_copy(oq, tp_o)
                    nc.sync.dma_start(out=out_d[128 * c:128 * (c + 1), :], in_=oq)

    nc.finalize()
    return nc


_CACHE = {}


def _get_nc():
    if "nc" not in _CACHE:
        _CACHE["nc"] = build_nc(S_FULL)
    return _CACHE["nc"]


def run(inputs, trace=False, tmpdir=None):
    from concourse.bass_utils import run_bass_kernel_spmd
    nc = _get_nc()
    arrs = {k: np.asarray(v, dtype=np.float32) for k, v in inputs.items()}
    shared = {k: np.ascontiguousarray(arrs[k]) for k in
              ("Wq", "bq", "Wk", "bk", "Wv", "bv", "gamma", "beta",
               "lam", "lambda_init")}
    in_maps = []
    for i in range(B):
        m = dict(shared)
        m["query"] = np.ascontiguousarray(arrs["query"][i])
        m["key"] = np.ascontiguousarray(arrs["key"][i])
        m["values"] = np.ascontiguousarray(arrs["values"][i])
        in_maps.append(m)
    res = run_bass_kernel_spmd(nc, in_maps, core_ids=list(range(B)),
                               trace=trace, tmpdir=tmpdir)
    out = np.stack([res.results[i]["out"] for i in range(B)], axis=0)
    return out.astype(np.float32), res


def kernel(**inputs):
    out, _ = run(inputs)
    return out
